# revision 46
# baseline (speedup 1.0000x reference)
"""GCN forward (2x graph-conv + global max-pool + linear) on 8 TRN2 NeuronCores.

Reference computation (N=16384 nodes, 256 feats, 64 hid):
    h1 = relu(adj @ (x @ W1) + b1)          [N, 64]
    h2 = adj @ (h1 @ W2) + b2               [N, 2]
    out = max(h2, axis=0) @ W3.T + b3       [1, 1, 1]

Distribution: row-shard adj over the 8 cores (core c owns output rows
[c*2048, (c+1)*2048)).  Each core:
  stage 1: Delta = bf16(16x)@bf16(W1) - m2, stored fp8  (replicated, tiny)
  pass A : h1T' = Delta.T @ adjT_fp8 + mt.T@rsum        [64, 2048] scaled
           bias/relu fused on psum evacuation (exact descale via act scale)
  stage 3: g_local = h1 @ W2 (fp32); delta_g = g_local - c
  AllGather delta_g -> delta_g_full [N, 2] (on-device collective, 64KB)
  pass B : h2T' = delta_g.T @ adjT_fp8 + ct.T@rsum      4x column-packed in
           one PSUM bank via tile_position (M=2 matmuls waste the array
           otherwise); per-i-chunk max -> [128, 1] per-core output
Host: unpack/max over strips and cores, + b2, @ W3.T + b3.

The adjacency streams as fp8e4m3 (x2^sa so max < 240), ONCE PER PASS =
64 MiB/core total.  fp8 noise is harmless because both passes compute the
large mean component exactly in fp32 via host-side sidecars:
  - rsum: exact f32 row-sums of adj (the only O(N^2) host work),
  - mt/ct: column-means, with exact cancellation of every quantization
    systematic (host simulates the device's bf16/fp8 quantization exactly,
    so the correction term absorbs the bias; only sqrt(N)-damped random
    noise survives).

Measured on trn2 (8 cores): HW exec ~308 us, rel-err vs fp32 ref 2.3e-4.
DMA roofline: 64 MiB adj + 8 MiB x + small, at ~350 GB/s/core -> ~210 us;
PE ~100 us (column-packed), hidden under the DMA stream; the residue is
the AllGather latency hole (~45 us, partially covered by 19-deep adj
prefetch) plus head/tail ramps.
"""

import os
import sys

sys.path.insert(0, "/opt/trn_rl_repo")

import numpy as np
import ml_dtypes


def _install_ntff_hook_shim():
    """The image's `antenv` lacks `axon_hooks`, which bass_utils imports for
    trace=True under axon. Provide it, wired to the PJRT .so's NRT-profile
    C ABI (same thing trn_boot would have registered)."""
    import types
    if "antenv.axon_hooks" in sys.modules:
        return
    try:
        import antenv  # noqa: F401
        from trn_agent_boot.trn_boot import _ntff_profile_via_ctypes
        mod = types.ModuleType("antenv.axon_hooks")
        _state = {"hook": _ntff_profile_via_ctypes("/opt/axon/libaxon_pjrt.so")}
        mod.set_axon_ntff_profile_hook = lambda h: _state.update(hook=h)
        mod.get_axon_ntff_profile_hook = lambda: _state["hook"]
        sys.modules["antenv.axon_hooks"] = mod
    except Exception:
        pass


_install_ntff_hook_shim()

import concourse.bass as bass
import concourse.mybir as mybir
import concourse.tile as tile
from concourse import bacc
from concourse.bass_utils import run_bass_kernel_spmd

BF16_NP = ml_dtypes.bfloat16
FP8_NP = ml_dtypes.float8_e4m3

P = 128          # partition dim
N_CORES = 8
N_NODES = 16384
N_FEAT = 256
N_HID = 64


class Cfg:
    def __init__(self, n=N_NODES, n_feat=N_FEAT, n_hid=N_HID, n_cores=N_CORES,
                 iw=512, kpg=16, mpg=8, adj_bufs=19, xt_bufs=2,
                 sa=21, sd=10, sx=4):
        self.n, self.n_feat, self.n_hid, self.n_cores = n, n_feat, n_hid, n_cores
        self.rows = n // n_cores       # output rows per core
        self.iw = iw                   # i-tile width (psum free dim)
        self.kpg = kpg                 # k-chunks (128 nodes each) per adj DMA
        self.mpg = mpg                 # m-chunks per xt DMA
        self.kc = n // P               # contraction chunks (over all nodes)
        self.nkg = self.kc // kpg      # adj DMA groups per i-chunk
        self.ni = self.rows // iw      # i-chunks per core
        self.mc = n // P               # stage-1 m-chunks (all nodes)
        self.nmg = self.mc // mpg      # xt DMA groups
        self.mcl = self.rows // P      # stage-3 m-chunks (local rows)
        self.fkc = n_feat // P         # feature contraction chunks
        self.adj_bufs = adj_bufs
        self.xt_bufs = xt_bufs
        # fp8 scales (powers of 2, exact): adj x2^sa keeps max < 240;
        # stage-1 operands x2^sx so Delta_fp8 is 2^sx-scaled; pass-B delta
        # x2^sd on device.  psA holds 2^(sa+sx)*h1T', psB 2^(sa+sd)*h2T'.
        self.sa = sa
        self.sd = sd
        self.sx = sx
        assert self.rows % iw == 0 and self.kc % kpg == 0 and self.mc % mpg == 0
        assert self.iw % P == 0 and self.ni <= 4
        assert self.ni in (1, 2, 4)
        assert self.kpg * self.iw <= 8192


def build_nc(cfg: Cfg) -> bass.Bass:
    BF = mybir.dt.bfloat16
    F32 = mybir.dt.float32
    FP8 = mybir.dt.float8e4
    n_hid, iw, kpg, fkc = cfg.n_hid, cfg.iw, cfg.kpg, cfg.fkc

    nc = bacc.Bacc("TRN2", target_bir_lowering=False)
    # adjt[n_i, kg][p, kl*iw + ii] = 2^sa * adjT_shard[128*(kg*kpg+kl)+p,
    # iw*n_i+ii] in fp8e4m3; streamed once per pass.
    adjt_h = nc.declare_dram_parameter(
        "adjt2", [cfg.ni, cfg.nkg, P, kpg * iw], FP8, isOutput=False)
    # xt[mg][p, (ml*fkc+k)*128 + c] = fp8(2 * x)[128*(mg*mpg+ml)+c, 128*k+p]
    # w1 = fp8(8 * W1); stage-1 product is 2^sx(=4)-scaled, fp8 operands get
    # FWL weight loads (stage-1 was LDW-bound in bf16)
    xt_h = nc.declare_dram_parameter(
        "xt", [cfg.nmg, P, cfg.mpg * fkc * P], FP8, isOutput=False)
    w1_h = nc.declare_dram_parameter("w1", [fkc, P, n_hid], FP8, isOutput=False)
    b1_h = nc.declare_dram_parameter("b1", [2 * n_hid, 1], F32, isOutput=False)
    w2_h = nc.declare_dram_parameter("w2", [2 * n_hid, 2], F32, isOutput=False)
    # host-side exactness sidecars (see module docstring):
    #   m2  = col-means of the device product bf16(2^sx x)@bf16(W1)  [scaled]
    #   mt  = (true col-means of x@W1 minus fp8(Delta) quantization bias)
    #         * 2^(sa+sx)   -- pass-A correction lhsT
    #   c2/ct = pass-B center estimate (c2 plain, ct * 2^(sa+sd))
    #   rsum  = exact f32 row-sums of this core's adj rows
    c2_h = nc.declare_dram_parameter("c2", [P, 2], F32, isOutput=False)
    ct_h = nc.declare_dram_parameter("ct", [1, 2], F32, isOutput=False)
    m2_h = nc.declare_dram_parameter("m2", [P, n_hid], F32, isOutput=False)
    mt_h = nc.declare_dram_parameter("mt", [1, n_hid], F32, isOutput=False)
    rs_h = nc.declare_dram_parameter("rsum", [1, cfg.rows], F32, isOutput=False)
    # out[32j + t] = max over i-chunk j (valid for j < ni, t < 2)
    out_h = nc.declare_dram_parameter("out", [P, 1], F32, isOutput=True)

    # collective bounce buffers: g_in[p, 2*m+t] = delta_g_local[128*m+p, t]
    g_in = nc.dram_tensor("g_in", [P, 2 * cfg.mcl], F32)
    g_out = nc.dram_tensor(
        "g_out", [P * cfg.n_cores, 2 * cfg.mcl], F32, addr_space="Shared")

    with tile.TileContext(nc, num_cores=cfg.n_cores) as tc:
        with (
            tc.tile_pool(name="const", bufs=1) as const_pool,
            tc.tile_pool(name="xw1p", bufs=1) as xw1_pool,
            tc.tile_pool(name="h1tp", bufs=1) as h1t_pool,
            tc.tile_pool(name="xtp", bufs=cfg.xt_bufs) as xt_pool,
            tc.tile_pool(name="adjp", bufs=cfg.adj_bufs) as adj_pool,
            tc.tile_pool(name="gp", bufs=1) as g_pool,
            tc.tile_pool(name="mxp", bufs=1) as mx_pool,
            tc.tile_pool(name="ps1p", bufs=2, space="PSUM") as ps1_pool,
            tc.tile_pool(name="psAp", bufs=2, space="PSUM") as psA_pool,
            tc.tile_pool(name="ps3p", bufs=2, space="PSUM") as ps3_pool,
            tc.tile_pool(name="psBp", bufs=1, space="PSUM") as psB_pool,
        ):
            # ---- constants to SBUF
            w1_sb = const_pool.tile([P, fkc * n_hid], FP8)
            for k in range(fkc):
                nc.sync.dma_start(
                    out=w1_sb[:, k * n_hid:(k + 1) * n_hid], in_=w1_h[k])
            b1_sb = const_pool.tile([2 * n_hid, 1], F32)
            nc.sync.dma_start(out=b1_sb[:, :], in_=b1_h[:, :])
            w2_sb = const_pool.tile([2 * n_hid, 2], F32)
            nc.sync.dma_start(out=w2_sb[:, :], in_=w2_h[:, :])
            c2_sb = const_pool.tile([P, 2], F32)
            nc.sync.dma_start(out=c2_sb[:, :], in_=c2_h[:, :])
            ct_sb = const_pool.tile([1, 2], F32)
            nc.sync.dma_start(out=ct_sb[:, :], in_=ct_h[:, :])
            m2_sb = const_pool.tile([P, n_hid], F32)
            nc.sync.dma_start(out=m2_sb[:, :], in_=m2_h[:, :])
            mt_sb = const_pool.tile([1, n_hid], F32)
            nc.sync.dma_start(out=mt_sb[:, :], in_=mt_h[:, :])
            rs_sb = const_pool.tile([1, cfg.rows], F32)
            nc.sync.dma_start(out=rs_sb[:, :], in_=rs_h[:, :])

            # ---- stage 1: Delta = (2^sx x)@W1 - m2, stored fp8 node-major
            xw1_sb = xw1_pool.tile([P, cfg.mc * n_hid], FP8)
            for mg in range(cfg.nmg):
                xt_t = xt_pool.tile([P, cfg.mpg * fkc * P], FP8, tag="xt")
                nc.sync.dma_start(out=xt_t[:, :], in_=xt_h[mg])
                for ml in range(cfg.mpg):
                    m = mg * cfg.mpg + ml
                    ps1 = ps1_pool.tile([P, n_hid], F32, tag="ps1")
                    for k in range(fkc):
                        nc.tensor.matmul(
                            ps1[:, :],
                            lhsT=xt_t[:, (ml * fkc + k) * P:(ml * fkc + k + 1) * P],
                            rhs=w1_sb[:, k * n_hid:(k + 1) * n_hid],
                            start=(k == 0), stop=(k == fkc - 1),
                        )
                    nc.vector.tensor_sub(
                        xw1_sb[:, m * n_hid:(m + 1) * n_hid], ps1[:, :],
                        m2_sb[:, :])

            # ---- pass A: 2^(sa+sx) h1T' = Delta.T @ adjT_fp8 + mt.T @ rsum
            # ---- stage 3: delta_g = h1 @ W2 - c (fp32, per i-chunk)
            # h1t[64s + h, a*iw + ii] = h1 for i-chunk (2a+s) (strip s in
            # array columns [64s, 64s+64), both strips share one psum bank)
            npair = max(1, cfg.ni // 2)
            nstrip = min(2, cfg.ni)
            h1t_sb = h1t_pool.tile([nstrip * n_hid, npair * iw], F32)
            gl_sb = g_pool.tile([P, 2 * cfg.mcl], F32)
            for a in range(npair):
                psA = psA_pool.tile([nstrip * n_hid, iw], F32, tag="psA")
                for kg in range(cfg.nkg):
                    ats = []
                    for s in range(nstrip):
                        at = adj_pool.tile([P, kpg * iw], FP8, tag="at")
                        nc.sync.dma_start(
                            out=at[:, :], in_=adjt_h[nstrip * a + s, kg])
                        ats.append(at)
                    for kl in range(kpg):
                        k = kg * kpg + kl
                        for s in range(nstrip):
                            nc.tensor.matmul(
                                psA[s * n_hid:(s + 1) * n_hid, :],
                                lhsT=xw1_sb[:, k * n_hid:(k + 1) * n_hid],
                                rhs=ats[s][:, kl * iw:(kl + 1) * iw],
                                start=(k == 0), stop=False,
                                tile_position=(0, s * n_hid),
                                skip_group_check=True,
                            )
                for s in range(nstrip):
                    nc.tensor.matmul(
                        psA[s * n_hid:(s + 1) * n_hid, :],
                        lhsT=mt_sb[:, :],
                        rhs=rs_sb[:, (nstrip * a + s) * iw:(nstrip * a + s + 1) * iw],
                        start=False, stop=True,
                        tile_position=(0, s * n_hid),
                        skip_group_check=True,
                    )
                # h1 = relu(2^-(sa+sx) * psA + b1), exact descale in fp32
                nc.scalar.activation(
                    h1t_sb[:, a * iw:(a + 1) * iw], psA[:, :],
                    mybir.ActivationFunctionType.Relu,
                    bias=b1_sb[:nstrip * n_hid, :],
                    scale=float(2.0 ** -(cfg.sa + cfg.sx)),
                )
                for s in range(nstrip):
                    for ml in range(iw // P):
                        m = (nstrip * a + s) * (iw // P) + ml
                        ps3 = ps3_pool.tile([P, 2], F32, tag="ps3")
                        nc.tensor.matmul(
                            ps3[:, :],
                            lhsT=h1t_sb[s * n_hid:(s + 1) * n_hid,
                                        a * iw + ml * P:a * iw + (ml + 1) * P],
                            rhs=w2_sb[s * n_hid:(s + 1) * n_hid, :],
                            start=True, stop=True,
                        )
                        nc.vector.tensor_sub(
                            gl_sb[:, 2 * m:2 * m + 2], ps3[:, :], c2_sb[:, :])
            nc.sync.dma_start(out=g_in[:, :], in_=gl_sb[:, :])

            # ---- AllGather delta_g across the 8 cores (HBM bounce buffers)
            nc.gpsimd.collective_compute(
                "AllGather", mybir.AluOpType.bypass,
                ins=[g_in[:, :]], outs=[g_out[:, :]],
                replica_groups=[list(range(cfg.n_cores))],
            )
            # g_out[(r*128+p), 2*m+t] -> node-major g_sb[p, 2*(r*mcl+m)+t]
            # NOTE: on the scalar (ACT) HWDGE queue, NOT sync — this DMA waits
            # on the collective, and putting it on the sync queue would stall
            # every pass-B prefetch DMA queued behind it on that sequencer.
            gf_sb = g_pool.tile([P, 2 * cfg.kc], F32)
            nc.scalar.dma_start(
                out=gf_sb[:, :].rearrange("p (r c) -> p r c", r=cfg.n_cores),
                in_=g_out[:, :].rearrange("(r p) c -> p r c", p=P))
            g_sb = g_pool.tile([P, 2 * cfg.kc], FP8)
            nc.scalar.activation(
                g_sb[:, :], gf_sb[:, :],
                mybir.ActivationFunctionType.Copy, scale=float(2 ** cfg.sd))

            # ---- pass B: all ni i-chunks packed into ONE [128, iw] psum bank
            # via PE column-tiling: strip j (array cols [32j, 32j+32)) computes
            # i-chunk j.  2^(sa+sd) h2T'[t, i] lands at psum[32j + t, ii].
            psB = psB_pool.tile([P, iw], F32)
            for kg in range(cfg.nkg):
                ats = []
                for n_i in range(cfg.ni):
                    at = adj_pool.tile([P, kpg * iw], FP8, tag="at")
                    nc.sync.dma_start(out=at[:, :], in_=adjt_h[n_i, kg])
                    ats.append(at)
                for kl in range(kpg):
                    k = kg * kpg + kl
                    for n_i in range(cfg.ni):
                        nc.tensor.matmul(
                            psB[32 * n_i:32 * n_i + 2, :],
                            lhsT=g_sb[:, 2 * k:2 * (k + 1)],
                            rhs=ats[n_i][:, kl * iw:(kl + 1) * iw],
                            start=(k == 0), stop=False,
                            tile_position=(0, 32 * n_i),
                            skip_group_check=True,
                        )
            for n_i in range(cfg.ni):
                nc.tensor.matmul(
                    psB[32 * n_i:32 * n_i + 2, :],
                    lhsT=ct_sb[:, :],
                    rhs=rs_sb[:, n_i * iw:(n_i + 1) * iw],
                    start=False, stop=True,
                    tile_position=(0, 32 * n_i),
                    skip_group_check=True,
                )
            # per-strip max over the free axis, partition-aligned
            mxsb = mx_pool.tile([P, 1], F32)
            nc.vector.memset(mxsb[:, :], 0.0)
            for n_i in range(cfg.ni):
                nc.vector.reduce_max(
                    mxsb[32 * n_i:32 * n_i + 2, :],
                    psB[32 * n_i:32 * n_i + 2, :], axis=mybir.AxisListType.X)
            mxo = mx_pool.tile([P, 1], F32)
            nc.scalar.mul(mxo[:, :], mxsb[:, :], float(2.0 ** -(cfg.sa + cfg.sd)))
            nc.sync.dma_start(out=out_h[:, :], in_=mxo[:, :])
    nc.compile()
    return nc


def shard_inputs(cfg: Cfg, x, adj, W1, b1, W2):
    """Host-side prep: pre-tile + quantize, and build the exactness sidecars
    (see module docstring)."""
    x = np.asarray(x, dtype=np.float32)
    adj = np.asarray(adj, dtype=np.float32)

    sxf = np.float32(2.0 ** cfg.sx)
    # xt[mg, p, ml, k, c] = fp8(2x)[128*(mg*mpg+ml)+c, 128*k+p]; w1 = fp8(8 W1)
    # (split scales keep both operands in fp8's normal range; product = 2^sx x@W1)
    xb = (x * np.float32(2.0)).astype(FP8_NP)
    assert np.isfinite(xb.astype(np.float32)).all()
    xt = xb.reshape(cfg.nmg, cfg.mpg, P, cfg.fkc, P).transpose(0, 4, 1, 3, 2)
    xt = np.ascontiguousarray(xt).reshape(cfg.nmg, P, cfg.mpg * cfg.fkc * P)

    W1f = np.asarray(W1, dtype=np.float32)
    b1f = np.asarray(b1, dtype=np.float32)
    W2f = np.asarray(W2, dtype=np.float32)
    w1b = (W1f * np.float32(8.0)).astype(FP8_NP)
    assert np.isfinite(w1b.astype(np.float32)).all()
    w1 = np.ascontiguousarray(w1b.reshape(cfg.fkc, P, cfg.n_hid))
    # b1/W2 duplicated into both partition halves for the pass-A 2x packing
    b1d = np.ascontiguousarray(
        np.concatenate([b1f, b1f]).reshape(2 * cfg.n_hid, 1))
    w2 = np.ascontiguousarray(np.vstack([W2f, W2f]))

    # --- pass-A sidecars: exact simulation of the device quantizations.
    # device stage-1 product (2^sx-scaled), bf16 operands, f32 accumulate:
    xW1_dev = xb.astype(np.float32) @ w1b.astype(np.float32)     # 2^sx-scaled
    m_dev = xW1_dev.mean(axis=0, dtype=np.float64).astype(np.float32)
    Q = xW1_dev - m_dev                                          # device Delta
    Qq = Q.astype(FP8_NP).astype(np.float32)                     # fp8(Delta)
    assert np.isfinite(Qq).all(), "Delta overflows fp8 range"
    eps = (Qq - Q).mean(axis=0, dtype=np.float64).astype(np.float32)
    m_true = (x.mean(axis=0, dtype=np.float64).astype(np.float32) @ W1f)
    # correction lhsT: in 2^(sa+sx)-scaled psum units per unit rowsum
    mt_val = (m_true * sxf - eps) * np.float32(2.0 ** cfg.sa)
    m2 = np.ascontiguousarray(
        np.broadcast_to(m_dev, (P, cfg.n_hid)).astype(np.float32))
    mt = np.ascontiguousarray(mt_val.reshape(1, cfg.n_hid).astype(np.float32))

    # --- pass-B center estimate from a row subsample (any c is exact;
    # closer c => smaller |delta_g| => less fp8 noise)
    idx = np.arange(0, cfg.n, max(1, cfg.n // 256))
    g_sub = np.maximum(adj[idx] @ (xW1_dev / sxf) + b1f, 0.0) @ W2f
    c_est = g_sub.mean(axis=0).astype(np.float32)                # [2]
    c2 = np.ascontiguousarray(np.broadcast_to(c_est, (P, 2)).astype(np.float32))
    ct = np.ascontiguousarray(
        (c_est * np.float32(2.0 ** (cfg.sa + cfg.sd))).reshape(1, 2))
    rsum = adj.sum(axis=1, dtype=np.float64).astype(np.float32)  # [n]

    saf = np.float32(2.0 ** cfg.sa)
    in_maps = []
    for c in range(cfg.n_cores):
        shard = adj[c * cfg.rows:(c + 1) * cfg.rows, :]
        # a[n_i, kg, p, kl, ii] = shard[iw*n_i+ii, 128*(kg*kpg+kl)+p]
        a5 = shard.reshape(cfg.ni, cfg.iw, cfg.nkg, cfg.kpg, P).transpose(0, 2, 4, 3, 1)
        a2 = np.ascontiguousarray((a5 * saf).astype(FP8_NP)).reshape(
            cfg.ni, cfg.nkg, P, cfg.kpg * cfg.iw)
        rs = np.ascontiguousarray(
            rsum[c * cfg.rows:(c + 1) * cfg.rows].reshape(1, cfg.rows))
        in_maps.append({"adjt2": a2, "xt": xt, "w1": w1, "b1": b1d,
                        "w2": w2, "c2": c2, "ct": ct, "m2": m2, "mt": mt,
                        "rsum": rs})
    return in_maps


def finish_on_host(cfg: Cfg, per_core_out, b2, W3, b3):
    """per_core_out: [n_cores, 128] device outputs (strip j's maxima at
    [32j + t]) -> [1,1,1] final output."""
    b2 = np.asarray(b2, dtype=np.float32)
    W3 = np.asarray(W3, dtype=np.float32)
    b3 = np.asarray(b3, dtype=np.float32)
    strips = np.stack([per_core_out[:, 32 * j:32 * j + 2]
                       for j in range(cfg.ni)])          # [ni, n_cores, 2]
    pooled = strips.max(axis=(0, 1)).astype(np.float32) + b2       # [2]
    out = pooled[None, None, :] @ W3.T + b3                        # [1,1,1]
    return out.astype(np.float32)


_NC_CACHE: dict = {}
LAST_RESULT = None  # BassKernelResults of the most recent run (for test.py)


def kernel(x, adj, W1, b1, W2, b2, W3, b3):
    cfg = Cfg()
    x = np.asarray(x)
    assert x.shape == (cfg.n, cfg.n_feat), x.shape
    if "nc" not in _NC_CACHE:
        _NC_CACHE["nc"] = build_nc(cfg)
    nc = _NC_CACHE["nc"]

    in_maps = shard_inputs(cfg, x, adj, W1, b1, W2)
    trace = os.environ.get("GCN_TRACE", "0") == "1"
    res = run_bass_kernel_spmd(
        nc, in_maps, core_ids=list(range(cfg.n_cores)), trace=trace)
    global LAST_RESULT
    LAST_RESULT = res
    per_core = np.stack(
        [np.asarray(r["out"][:, 0], dtype=np.float32) for r in res.results])
    return finish_on_host(cfg, per_core, b2, W3, b3)


# revision 47
# speedup vs baseline: 1.0024x; 1.0024x over previous
"""GCN forward (2x graph-conv + global max-pool + linear) on 8 TRN2 NeuronCores.

Reference computation (N=16384 nodes, 256 feats, 64 hid):
    h1 = relu(adj @ (x @ W1) + b1)          [N, 64]
    h2 = adj @ (h1 @ W2) + b2               [N, 2]
    out = max(h2, axis=0) @ W3.T + b3       [1, 1, 1]

Distribution: row-shard adj over the 8 cores (core c owns output rows
[c*2048, (c+1)*2048)).  Each core:
  stage 1: Delta = bf16(16x)@bf16(W1) - m2, stored fp8  (replicated, tiny)
  pass A : h1T' = Delta.T @ adjT_fp8 + mt.T@rsum        [64, 2048] scaled
           bias/relu fused on psum evacuation (exact descale via act scale)
  stage 3: g_local = h1 @ W2 (fp32); delta_g = g_local - c
  AllGather delta_g -> delta_g_full [N, 2] (on-device collective, 64KB)
  pass B : h2T' = delta_g.T @ adjT_fp8 + ct.T@rsum      4x column-packed in
           one PSUM bank via tile_position (M=2 matmuls waste the array
           otherwise); per-i-chunk max -> [128, 1] per-core output
Host: unpack/max over strips and cores, + b2, @ W3.T + b3.

The adjacency streams as fp8e4m3 (x2^sa so max < 240), ONCE PER PASS =
64 MiB/core total.  fp8 noise is harmless because both passes compute the
large mean component exactly in fp32 via host-side sidecars:
  - rsum: exact f32 row-sums of adj (the only O(N^2) host work),
  - mt/ct: column-means, with exact cancellation of every quantization
    systematic (host simulates the device's bf16/fp8 quantization exactly,
    so the correction term absorbs the bias; only sqrt(N)-damped random
    noise survives).

Measured on trn2 (8 cores): HW exec ~308 us, rel-err vs fp32 ref 2.3e-4.
DMA roofline: 64 MiB adj + 8 MiB x + small, at ~350 GB/s/core -> ~210 us;
PE ~100 us (column-packed), hidden under the DMA stream; the residue is
the AllGather latency hole (~45 us, partially covered by 19-deep adj
prefetch) plus head/tail ramps.
"""

import os
import sys

sys.path.insert(0, "/opt/trn_rl_repo")

import numpy as np
import ml_dtypes


def _install_ntff_hook_shim():
    """The image's `antenv` lacks `axon_hooks`, which bass_utils imports for
    trace=True under axon. Provide it, wired to the PJRT .so's NRT-profile
    C ABI (same thing trn_boot would have registered)."""
    import types
    if "antenv.axon_hooks" in sys.modules:
        return
    try:
        import antenv  # noqa: F401
        from trn_agent_boot.trn_boot import _ntff_profile_via_ctypes
        mod = types.ModuleType("antenv.axon_hooks")
        _state = {"hook": _ntff_profile_via_ctypes("/opt/axon/libaxon_pjrt.so")}
        mod.set_axon_ntff_profile_hook = lambda h: _state.update(hook=h)
        mod.get_axon_ntff_profile_hook = lambda: _state["hook"]
        sys.modules["antenv.axon_hooks"] = mod
    except Exception:
        pass


_install_ntff_hook_shim()

import concourse.bass as bass
import concourse.mybir as mybir
import concourse.tile as tile
from concourse import bacc
from concourse.bass_utils import run_bass_kernel_spmd

BF16_NP = ml_dtypes.bfloat16
FP8_NP = ml_dtypes.float8_e4m3

P = 128          # partition dim
N_CORES = 8
N_NODES = 16384
N_FEAT = 256
N_HID = 64


class Cfg:
    def __init__(self, n=N_NODES, n_feat=N_FEAT, n_hid=N_HID, n_cores=N_CORES,
                 iw=512, kpg=16, mpg=8, adj_bufs=19, xt_bufs=2,
                 sa=21, sd=10, sx=4):
        self.n, self.n_feat, self.n_hid, self.n_cores = n, n_feat, n_hid, n_cores
        self.rows = n // n_cores       # output rows per core
        self.iw = iw                   # i-tile width (psum free dim)
        self.kpg = kpg                 # k-chunks (128 nodes each) per adj DMA
        self.mpg = mpg                 # m-chunks per xt DMA
        self.kc = n // P               # contraction chunks (over all nodes)
        self.nkg = self.kc // kpg      # adj DMA groups per i-chunk
        self.ni = self.rows // iw      # i-chunks per core
        self.mc = n // P               # stage-1 m-chunks (all nodes)
        self.nmg = self.mc // mpg      # xt DMA groups
        self.mcl = self.rows // P      # stage-3 m-chunks (local rows)
        self.fkc = n_feat // P         # feature contraction chunks
        self.adj_bufs = adj_bufs
        self.xt_bufs = xt_bufs
        # fp8 scales (powers of 2, exact): adj x2^sa keeps max < 240;
        # stage-1 operands x2^sx so Delta_fp8 is 2^sx-scaled; pass-B delta
        # x2^sd on device.  psA holds 2^(sa+sx)*h1T', psB 2^(sa+sd)*h2T'.
        self.sa = sa
        self.sd = sd
        self.sx = sx
        assert self.rows % iw == 0 and self.kc % kpg == 0 and self.mc % mpg == 0
        assert self.iw % P == 0 and self.ni <= 4
        assert self.ni in (1, 2, 4)
        assert self.kpg * self.iw <= 8192


def build_nc(cfg: Cfg) -> bass.Bass:
    BF = mybir.dt.bfloat16
    F32 = mybir.dt.float32
    FP8 = mybir.dt.float8e4
    n_hid, iw, kpg, fkc = cfg.n_hid, cfg.iw, cfg.kpg, cfg.fkc

    nc = bacc.Bacc("TRN2", target_bir_lowering=False)
    # adjt[n_i, kg][p, kl*iw + ii] = 2^sa * adjT_shard[128*(kg*kpg+kl)+p,
    # iw*n_i+ii] in fp8e4m3; streamed once per pass.
    adjt_h = nc.declare_dram_parameter(
        "adjt2", [cfg.ni, cfg.nkg, P, kpg * iw], FP8, isOutput=False)
    # xt[mg][p, (ml*fkc+k)*128 + c] = bf16(2^sx * x)[128*(mg*mpg+ml)+c, 128*k+p]
    xt_h = nc.declare_dram_parameter(
        "xt", [cfg.nmg, P, cfg.mpg * fkc * P], BF, isOutput=False)
    w1_h = nc.declare_dram_parameter("w1", [fkc, P, n_hid], BF, isOutput=False)
    b1_h = nc.declare_dram_parameter("b1", [2 * n_hid, 1], F32, isOutput=False)
    w2_h = nc.declare_dram_parameter("w2", [2 * n_hid, 2], F32, isOutput=False)
    # host-side exactness sidecars (see module docstring):
    #   m2  = col-means of the device product bf16(2^sx x)@bf16(W1)  [scaled]
    #   mt  = (true col-means of x@W1 minus fp8(Delta) quantization bias)
    #         * 2^(sa+sx)   -- pass-A correction lhsT
    #   c2/ct = pass-B center estimate (c2 plain, ct * 2^(sa+sd))
    #   rsum  = exact f32 row-sums of this core's adj rows
    c2_h = nc.declare_dram_parameter("c2", [P, 2], F32, isOutput=False)
    ct_h = nc.declare_dram_parameter("ct", [1, 2], F32, isOutput=False)
    m2_h = nc.declare_dram_parameter("m2", [P, n_hid], F32, isOutput=False)
    mt_h = nc.declare_dram_parameter("mt", [1, n_hid], F32, isOutput=False)
    rs_h = nc.declare_dram_parameter("rsum", [1, cfg.rows], F32, isOutput=False)
    # out[32j + t] = max over i-chunk j (valid for j < ni, t < 2)
    out_h = nc.declare_dram_parameter("out", [P, 1], F32, isOutput=True)

    # collective bounce buffers: g_in[p, 2*m+t] = delta_g_local[128*m+p, t]
    g_in = nc.dram_tensor("g_in", [P, 2 * cfg.mcl], F32)
    g_out = nc.dram_tensor(
        "g_out", [P * cfg.n_cores, 2 * cfg.mcl], F32, addr_space="Shared")

    with tile.TileContext(nc, num_cores=cfg.n_cores) as tc:
        with (
            tc.tile_pool(name="const", bufs=1) as const_pool,
            tc.tile_pool(name="xw1p", bufs=1) as xw1_pool,
            tc.tile_pool(name="h1tp", bufs=1) as h1t_pool,
            tc.tile_pool(name="xtp", bufs=cfg.xt_bufs) as xt_pool,
            tc.tile_pool(name="adjp", bufs=cfg.adj_bufs) as adj_pool,
            tc.tile_pool(name="gp", bufs=1) as g_pool,
            tc.tile_pool(name="mxp", bufs=1) as mx_pool,
            tc.tile_pool(name="ps1p", bufs=2, space="PSUM") as ps1_pool,
            tc.tile_pool(name="psAp", bufs=2, space="PSUM") as psA_pool,
            tc.tile_pool(name="ps3p", bufs=2, space="PSUM") as ps3_pool,
            tc.tile_pool(name="psBp", bufs=1, space="PSUM") as psB_pool,
        ):
            # ---- constants to SBUF
            w1_sb = const_pool.tile([P, fkc * n_hid], BF)
            for k in range(fkc):
                nc.sync.dma_start(
                    out=w1_sb[:, k * n_hid:(k + 1) * n_hid], in_=w1_h[k])
            b1_sb = const_pool.tile([2 * n_hid, 1], F32)
            nc.sync.dma_start(out=b1_sb[:, :], in_=b1_h[:, :])
            w2_sb = const_pool.tile([2 * n_hid, 2], F32)
            nc.sync.dma_start(out=w2_sb[:, :], in_=w2_h[:, :])
            c2_sb = const_pool.tile([P, 2], F32)
            nc.sync.dma_start(out=c2_sb[:, :], in_=c2_h[:, :])
            ct_sb = const_pool.tile([1, 2], F32)
            nc.sync.dma_start(out=ct_sb[:, :], in_=ct_h[:, :])
            m2_sb = const_pool.tile([P, n_hid], F32)
            nc.sync.dma_start(out=m2_sb[:, :], in_=m2_h[:, :])
            mt_sb = const_pool.tile([1, n_hid], F32)
            nc.sync.dma_start(out=mt_sb[:, :], in_=mt_h[:, :])
            rs_sb = const_pool.tile([1, cfg.rows], F32)
            nc.sync.dma_start(out=rs_sb[:, :], in_=rs_h[:, :])

            # ---- stage 1: Delta = (2^sx x)@W1 - m2, stored fp8 node-major
            xw1_sb = xw1_pool.tile([P, cfg.mc * n_hid], FP8)
            for mg in range(cfg.nmg):
                xt_t = xt_pool.tile([P, cfg.mpg * fkc * P], BF, tag="xt")
                nc.sync.dma_start(out=xt_t[:, :], in_=xt_h[mg])
                for ml in range(cfg.mpg):
                    m = mg * cfg.mpg + ml
                    ps1 = ps1_pool.tile([P, n_hid], F32, tag="ps1")
                    for k in range(fkc):
                        nc.tensor.matmul(
                            ps1[:, :],
                            lhsT=xt_t[:, (ml * fkc + k) * P:(ml * fkc + k + 1) * P],
                            rhs=w1_sb[:, k * n_hid:(k + 1) * n_hid],
                            start=(k == 0), stop=(k == fkc - 1),
                        )
                    nc.vector.tensor_sub(
                        xw1_sb[:, m * n_hid:(m + 1) * n_hid], ps1[:, :],
                        m2_sb[:, :])

            # ---- pass A: 2^(sa+sx) h1T' = Delta.T @ adjT_fp8 + mt.T @ rsum
            # ---- stage 3: delta_g = h1 @ W2 - c (fp32, per i-chunk)
            # h1t[64s + h, a*iw + ii] = h1 for i-chunk (2a+s) (strip s in
            # array columns [64s, 64s+64), both strips share one psum bank)
            npair = max(1, cfg.ni // 2)
            nstrip = min(2, cfg.ni)
            h1t_sb = h1t_pool.tile([nstrip * n_hid, npair * iw], F32)
            gl_sb = g_pool.tile([P, 2 * cfg.mcl], F32)
            for a in range(npair):
                psA = psA_pool.tile([nstrip * n_hid, iw], F32, tag="psA")
                for kg in range(cfg.nkg):
                    ats = []
                    for s in range(nstrip):
                        at = adj_pool.tile([P, kpg * iw], FP8, tag="at")
                        nc.sync.dma_start(
                            out=at[:, :], in_=adjt_h[nstrip * a + s, kg])
                        ats.append(at)
                    for kl in range(kpg):
                        k = kg * kpg + kl
                        for s in range(nstrip):
                            nc.tensor.matmul(
                                psA[s * n_hid:(s + 1) * n_hid, :],
                                lhsT=xw1_sb[:, k * n_hid:(k + 1) * n_hid],
                                rhs=ats[s][:, kl * iw:(kl + 1) * iw],
                                start=(k == 0), stop=False,
                                tile_position=(0, s * n_hid),
                                skip_group_check=True,
                            )
                for s in range(nstrip):
                    nc.tensor.matmul(
                        psA[s * n_hid:(s + 1) * n_hid, :],
                        lhsT=mt_sb[:, :],
                        rhs=rs_sb[:, (nstrip * a + s) * iw:(nstrip * a + s + 1) * iw],
                        start=False, stop=True,
                        tile_position=(0, s * n_hid),
                        skip_group_check=True,
                    )
                # h1 = relu(2^-(sa+sx) * psA + b1), exact descale in fp32
                nc.scalar.activation(
                    h1t_sb[:, a * iw:(a + 1) * iw], psA[:, :],
                    mybir.ActivationFunctionType.Relu,
                    bias=b1_sb[:nstrip * n_hid, :],
                    scale=float(2.0 ** -(cfg.sa + cfg.sx)),
                )
                for s in range(nstrip):
                    for ml in range(iw // P):
                        m = (nstrip * a + s) * (iw // P) + ml
                        ps3 = ps3_pool.tile([P, 2], F32, tag="ps3")
                        nc.tensor.matmul(
                            ps3[:, :],
                            lhsT=h1t_sb[s * n_hid:(s + 1) * n_hid,
                                        a * iw + ml * P:a * iw + (ml + 1) * P],
                            rhs=w2_sb[s * n_hid:(s + 1) * n_hid, :],
                            start=True, stop=True,
                        )
                        nc.vector.tensor_sub(
                            gl_sb[:, 2 * m:2 * m + 2], ps3[:, :], c2_sb[:, :])
            nc.sync.dma_start(out=g_in[:, :], in_=gl_sb[:, :])

            # ---- AllGather delta_g across the 8 cores (HBM bounce buffers)
            nc.gpsimd.collective_compute(
                "AllGather", mybir.AluOpType.bypass,
                ins=[g_in[:, :]], outs=[g_out[:, :]],
                replica_groups=[list(range(cfg.n_cores))],
            )
            # g_out[(r*128+p), 2*m+t] -> node-major g_sb[p, 2*(r*mcl+m)+t]
            # NOTE: on the scalar (ACT) HWDGE queue, NOT sync — this DMA waits
            # on the collective, and putting it on the sync queue would stall
            # every pass-B prefetch DMA queued behind it on that sequencer.
            gf_sb = g_pool.tile([P, 2 * cfg.kc], F32)
            nc.scalar.dma_start(
                out=gf_sb[:, :].rearrange("p (r c) -> p r c", r=cfg.n_cores),
                in_=g_out[:, :].rearrange("(r p) c -> p r c", p=P))
            g_sb = g_pool.tile([P, 2 * cfg.kc], FP8)
            nc.scalar.activation(
                g_sb[:, :], gf_sb[:, :],
                mybir.ActivationFunctionType.Copy, scale=float(2 ** cfg.sd))

            # ---- pass B: all ni i-chunks packed into ONE [128, iw] psum bank
            # via PE column-tiling: strip j (array cols [32j, 32j+32)) computes
            # i-chunk j.  2^(sa+sd) h2T'[t, i] lands at psum[32j + t, ii].
            psB = psB_pool.tile([P, iw], F32)
            for kg in range(cfg.nkg):
                ats = []
                for n_i in range(cfg.ni):
                    at = adj_pool.tile([P, kpg * iw], FP8, tag="at")
                    nc.sync.dma_start(out=at[:, :], in_=adjt_h[n_i, kg])
                    ats.append(at)
                for kl in range(kpg):
                    k = kg * kpg + kl
                    for n_i in range(cfg.ni):
                        nc.tensor.matmul(
                            psB[32 * n_i:32 * n_i + 2, :],
                            lhsT=g_sb[:, 2 * k:2 * (k + 1)],
                            rhs=ats[n_i][:, kl * iw:(kl + 1) * iw],
                            start=(k == 0), stop=False,
                            tile_position=(0, 32 * n_i),
                            skip_group_check=True,
                        )
            for n_i in range(cfg.ni):
                nc.tensor.matmul(
                    psB[32 * n_i:32 * n_i + 2, :],
                    lhsT=ct_sb[:, :],
                    rhs=rs_sb[:, n_i * iw:(n_i + 1) * iw],
                    start=False, stop=True,
                    tile_position=(0, 32 * n_i),
                    skip_group_check=True,
                )
            # per-strip max over the free axis, partition-aligned
            mxsb = mx_pool.tile([P, 1], F32)
            nc.vector.memset(mxsb[:, :], 0.0)
            for n_i in range(cfg.ni):
                nc.vector.reduce_max(
                    mxsb[32 * n_i:32 * n_i + 2, :],
                    psB[32 * n_i:32 * n_i + 2, :], axis=mybir.AxisListType.X)
            mxo = mx_pool.tile([P, 1], F32)
            nc.scalar.mul(mxo[:, :], mxsb[:, :], float(2.0 ** -(cfg.sa + cfg.sd)))
            nc.sync.dma_start(out=out_h[:, :], in_=mxo[:, :])
    nc.compile()
    return nc


def shard_inputs(cfg: Cfg, x, adj, W1, b1, W2):
    """Host-side prep: pre-tile + quantize, and build the exactness sidecars
    (see module docstring)."""
    x = np.asarray(x, dtype=np.float32)
    adj = np.asarray(adj, dtype=np.float32)

    sxf = np.float32(2.0 ** cfg.sx)
    # xt[mg, p, ml, k, c] = bf16(2^sx * x)[128*(mg*mpg+ml)+c, 128*k+p]
    xb = (x * sxf).astype(BF16_NP)
    xt = xb.reshape(cfg.nmg, cfg.mpg, P, cfg.fkc, P).transpose(0, 4, 1, 3, 2)
    xt = np.ascontiguousarray(xt).reshape(cfg.nmg, P, cfg.mpg * cfg.fkc * P)

    W1f = np.asarray(W1, dtype=np.float32)
    b1f = np.asarray(b1, dtype=np.float32)
    W2f = np.asarray(W2, dtype=np.float32)
    w1b = W1f.astype(BF16_NP)
    w1 = np.ascontiguousarray(w1b.reshape(cfg.fkc, P, cfg.n_hid))
    # b1/W2 duplicated into both partition halves for the pass-A 2x packing
    b1d = np.ascontiguousarray(
        np.concatenate([b1f, b1f]).reshape(2 * cfg.n_hid, 1))
    w2 = np.ascontiguousarray(np.vstack([W2f, W2f]))

    # --- pass-A sidecars: exact simulation of the device quantizations.
    # device stage-1 product (2^sx-scaled), bf16 operands, f32 accumulate:
    xW1_dev = xb.astype(np.float32) @ w1b.astype(np.float32)     # 2^sx-scaled
    m_dev = xW1_dev.mean(axis=0, dtype=np.float64).astype(np.float32)
    Q = xW1_dev - m_dev                                          # device Delta
    Qq = Q.astype(FP8_NP).astype(np.float32)                     # fp8(Delta)
    assert np.isfinite(Qq).all(), "Delta overflows fp8 range"
    eps = (Qq - Q).mean(axis=0, dtype=np.float64).astype(np.float32)
    m_true = (x.mean(axis=0, dtype=np.float64).astype(np.float32) @ W1f)
    # correction lhsT: in 2^(sa+sx)-scaled psum units per unit rowsum
    mt_val = (m_true * sxf - eps) * np.float32(2.0 ** cfg.sa)
    m2 = np.ascontiguousarray(
        np.broadcast_to(m_dev, (P, cfg.n_hid)).astype(np.float32))
    mt = np.ascontiguousarray(mt_val.reshape(1, cfg.n_hid).astype(np.float32))

    # --- pass-B center estimate from a row subsample (any c is exact;
    # closer c => smaller |delta_g| => less fp8 noise)
    idx = np.arange(0, cfg.n, max(1, cfg.n // 256))
    g_sub = np.maximum(adj[idx] @ (xW1_dev / sxf) + b1f, 0.0) @ W2f
    c_est = g_sub.mean(axis=0).astype(np.float32)                # [2]
    c2 = np.ascontiguousarray(np.broadcast_to(c_est, (P, 2)).astype(np.float32))
    ct = np.ascontiguousarray(
        (c_est * np.float32(2.0 ** (cfg.sa + cfg.sd))).reshape(1, 2))
    rsum = adj.sum(axis=1, dtype=np.float64).astype(np.float32)  # [n]

    saf = np.float32(2.0 ** cfg.sa)
    in_maps = []
    for c in range(cfg.n_cores):
        shard = adj[c * cfg.rows:(c + 1) * cfg.rows, :]
        # a[n_i, kg, p, kl, ii] = shard[iw*n_i+ii, 128*(kg*kpg+kl)+p]
        a5 = shard.reshape(cfg.ni, cfg.iw, cfg.nkg, cfg.kpg, P).transpose(0, 2, 4, 3, 1)
        a2 = np.ascontiguousarray((a5 * saf).astype(FP8_NP)).reshape(
            cfg.ni, cfg.nkg, P, cfg.kpg * cfg.iw)
        rs = np.ascontiguousarray(
            rsum[c * cfg.rows:(c + 1) * cfg.rows].reshape(1, cfg.rows))
        in_maps.append({"adjt2": a2, "xt": xt, "w1": w1, "b1": b1d,
                        "w2": w2, "c2": c2, "ct": ct, "m2": m2, "mt": mt,
                        "rsum": rs})
    return in_maps


def finish_on_host(cfg: Cfg, per_core_out, b2, W3, b3):
    """per_core_out: [n_cores, 128] device outputs (strip j's maxima at
    [32j + t]) -> [1,1,1] final output."""
    b2 = np.asarray(b2, dtype=np.float32)
    W3 = np.asarray(W3, dtype=np.float32)
    b3 = np.asarray(b3, dtype=np.float32)
    strips = np.stack([per_core_out[:, 32 * j:32 * j + 2]
                       for j in range(cfg.ni)])          # [ni, n_cores, 2]
    pooled = strips.max(axis=(0, 1)).astype(np.float32) + b2       # [2]
    out = pooled[None, None, :] @ W3.T + b3                        # [1,1,1]
    return out.astype(np.float32)


_NC_CACHE: dict = {}
LAST_RESULT = None  # BassKernelResults of the most recent run (for test.py)


def kernel(x, adj, W1, b1, W2, b2, W3, b3):
    cfg = Cfg()
    x = np.asarray(x)
    assert x.shape == (cfg.n, cfg.n_feat), x.shape
    if "nc" not in _NC_CACHE:
        _NC_CACHE["nc"] = build_nc(cfg)
    nc = _NC_CACHE["nc"]

    in_maps = shard_inputs(cfg, x, adj, W1, b1, W2)
    trace = os.environ.get("GCN_TRACE", "0") == "1"
    res = run_bass_kernel_spmd(
        nc, in_maps, core_ids=list(range(cfg.n_cores)), trace=trace)
    global LAST_RESULT
    LAST_RESULT = res
    per_core = np.stack(
        [np.asarray(r["out"][:, 0], dtype=np.float32) for r in res.results])
    return finish_on_host(cfg, per_core, b2, W3, b3)
